# revision 26
# baseline (speedup 1.0000x reference)
"""MultiHeadGAT layer on 8 trn2 NeuronCores, data-parallel over batch.

Per core (one batch element), exp(leaky_relu(e_src[i]+e_dst[j])) is
factored rank-1:  with u=exp(e_src), r=exp(-0.8 e_src), v=exp(e_dst),
z=exp(0.2 e_dst):

    exp(lrelu(s_ij)) = u_i * max(r_i z_j, v_j)

The row factor u_i cancels in the softmax, so the per-element work is

    S'[j,i] = adj[i,j] * max(r_i * z_j, v_j)

one fused DVE tensor_scalar (mult+max, fp16) + one DVE tensor_tensor
mask multiply (fp16 2x) per [128,1024] tile.  No full-size exp at all
(exp only on [8,1024] vectors; v and z share one exp via a host-side
WA extension).  The host passes h and adj pre-transposed, so no PE
transposes on the input side.  The AV matmul runs fp16 (1 cycle/row)
with a ones column appended to Wh so row 64 of the accumulator is the
softmax denominator.

r-broadcast tiles are produced by a transpose-mode matmul
(selector.T @ rv -> one fp16 PSUM bank) + one ACT copy, emitted two
heads ahead from inside the main loop so they slot into PE/ACT idle
gaps.  Epilogue is split: PSUM copies + transposes right after each
head's matmuls (fills PE gaps, keeps HAM warm); reciprocal + scaled
copies deferred one head so the DVE FIFO never stalls.  Output DMAs
go out in column halves after heads 3 and 7.
"""
import sys

sys.path.insert(0, "/opt/trn_rl_repo")

import numpy as np

import concourse.bass as bass
import concourse.mybir as mybir
import concourse.tile as tile
from concourse.bass_utils import run_bass_kernel_spmd
from concourse.masks import make_identity

F32 = mybir.dt.float32
FP16 = mybir.dt.float16
I32 = mybir.dt.int32
AF = mybir.ActivationFunctionType
ALU = mybir.AluOpType

N_CORES = 8
N = 1024
NB = 8          # row blocks of 128
FIN = 256
KT = 2          # FIN / 128
FO = 512        # heads * fo
H = 8
FOH = 64
ALPHA = 0.2
NPRE = 2        # heads whose r-broadcast is emitted before the main loop

_MAX_SYNC_WAITS = 1


def _split_sync_waits(nc, max_waits=_MAX_SYNC_WAITS):
    """This walrus build rejects instructions carrying more than one sync
    wait; hoist extras onto NOPs inserted just before, on the same engine."""
    uid = 0
    for f in nc.m.functions:
        for bb in f.blocks:
            out = []
            for inst in bb.instructions:
                si = getattr(inst, "sync_info", None)
                if si is not None and si.on_wait and len(si.on_wait) > max_waits:
                    waits = list(si.on_wait)
                    keep = waits[-max_waits:]
                    extra = waits[:-max_waits]
                    si.on_wait.clear()
                    si.on_wait.extend(keep)
                    while extra:
                        chunk, extra = extra[:max_waits], extra[max_waits:]
                        nop = mybir.InstNoOp(
                            name=f"waitsplit-{uid}",
                            engine=inst.engine,
                            sync_info=mybir.SyncInfo(
                                on_wait=list(chunk), on_update=[]
                            ),
                            bass_nofuse=True,
                        )
                        uid += 1
                        out.append(nop)
                out.append(inst)
            bb.instructions[:] = out


def build_nc(split=True):
    nc = bass.Bass()
    ht_d = nc.declare_dram_parameter("hT", [FIN, N], F32, isOutput=False)
    adjt_d = nc.declare_dram_parameter("adjT", [N, N], I32, isOutput=False)
    w_d = nc.declare_dram_parameter("W", [FIN, FO], F32, isOutput=False)
    es_d = nc.declare_dram_parameter("ES", [H, N], F32, isOutput=False)
    ed_d = nc.declare_dram_parameter("ED2", [2 * H, N], F32, isOutput=False)
    out_d = nc.declare_dram_parameter("out", [N, FO], F32, isOutput=True)

    with tile.TileContext(nc) as tc:
        with (
            tc.tile_pool(name="const", bufs=1) as const,
            tc.tile_pool(name="persist", bufs=1) as persist,
            tc.tile_pool(name="ld", bufs=4) as ld,
            tc.tile_pool(name="xp", bufs=8) as xp,
            tc.tile_pool(name="epi", bufs=2) as epi,
            tc.tile_pool(name="psS", bufs=2, space="PSUM") as psS,
            tc.tile_pool(name="psAcc", bufs=2, space="PSUM") as psAcc,
        ):
            ident = const.tile([128, 128], F32, tag="ident")
            make_identity(nc, ident[:])
            identh = const.tile([128, 128], FP16, tag="identh")
            nc.vector.tensor_copy(identh[:], ident[:])

            # ---- e vectors (host-computed h @ WA, tiny): load first ----
            e_src_t = const.tile([8, N], F32, tag="esT")
            ed2 = const.tile([16, N], F32, tag="ed2")
            nc.sync.dma_start(e_src_t[:], es_d[:, :])
            nc.sync.dma_start(ed2[:], ed_d[:, :])

            # ---- selector tiles for heads 0/1 before the adjB queue ----
            sel = []
            for hh in range(H):
                t = const.tile([8, 128], FP16, tag=f"sel{hh}", name=f"sel{hh}")
                sel.append(t)

            def mk_sel(hh):
                t = sel[hh]
                nc.gpsimd.memset(t[:], 0.0)
                nc.gpsimd.affine_select(
                    out=t[:], in_=t[:], pattern=[[0, 128]],
                    compare_op=ALU.not_equal, fill=1.0,
                    base=-hh, channel_multiplier=1,
                )

            for hh in range(NPRE):
                mk_sel(hh)

            # ---- adj.T (int32 DRAM) -> fp16 SBUF via SWDGE cast DMA ----
            adjB = [persist.tile([128, N], FP16, tag=f"adjB{j}",
                                 name=f"adjB{j}")
                    for j in range(NB)]
            for jb in range(NB):
                nc.gpsimd.dma_start(
                    adjB[jb][:], adjt_d[jb * 128:(jb + 1) * 128, :]
                )
            for hh in range(NPRE, H):
                mk_sel(hh)

            # ---- hT (fp32 DRAM, pre-transposed on host): sync load,
            # cast to fp16 on ACT (off the DVE FIFO) ----
            hT32 = [persist.tile([128, N], F32, tag=f"hT32_{k}",
                                 name=f"hT32_{k}")
                    for k in range(KT)]
            for k in range(KT):
                for c in range(2):
                    nc.sync.dma_start(
                        hT32[k][:, c * 512:(c + 1) * 512],
                        ht_d[k * 128:(k + 1) * 128, c * 512:(c + 1) * 512],
                    )
            wk32 = []
            for k in range(KT):
                t32 = ld.tile([128, FO], F32, tag="w32", name=f"w32_{k}")
                nc.sync.dma_start(t32[:], w_d[k * 128:(k + 1) * 128, :])
                wk32.append(t32)

            # ---- derived exp vectors ----
            # rv_t[hh, i] = exp(-0.8 * e_src[hh, i])        (fp16)
            rv_t = const.tile([8, N], FP16, tag="rvT")
            nc.scalar.activation(rv_t[:], e_src_t[:], AF.Exp, scale=-0.8)
            # vzexp rows 0:8 = v = exp(e_dst); rows 8:16 = z = exp(0.2 e_dst)
            vzexp = const.tile([16, N], FP16, tag="vzexp")
            nc.scalar.activation(vzexp[:], ed2[:], AF.Exp)

            # ---- vz_sb[jb][p, 0:8]=v_h(j), [p, 8:16]=z_h(j)  (f32) ----
            vz_sb = [persist.tile([128, 16], F32, tag=f"vz{j}", name=f"vz{j}")
                     for j in range(NB)]
            for jb in range(NB):
                tp = psS.tile([128, 512], FP16, tag="ps")
                nc.tensor.transpose(
                    tp[:, 0:16], vzexp[:, jb * 128:(jb + 1) * 128],
                    identh[0:16, 0:16],
                )
                nc.vector.tensor_copy(vz_sb[jb][:], tp[:, 0:16])

            # ---- r-broadcast: r_all[p, hh*N+i] = rv_t[hh, i].
            # One transpose-mode matmul (sel.T @ rv -> fp16 PSUM, one bank)
            # + one ACT copy per head.  Heads 0..NPRE-1 up front; head hh+2
            # emitted from inside head hh's loop body. ----
            r_all = persist.tile([128, H * N], FP16, tag="rall")

            def bcast(hh):
                for c in range(2):
                    rb = psS.tile([128, 512], F32, tag="ps", name="rb")
                    nc.tensor.matmul(
                        rb[:], sel[hh][:], rv_t[:, c * 512:(c + 1) * 512],
                        start=True, stop=True,
                    )
                    nc.scalar.copy(
                        r_all[:, hh * N + c * 512:hh * N + (c + 1) * 512],
                        rb[:],
                    )

            for hh in range(NPRE):
                bcast(hh)

            # ---- hT / W fp16 casts on ACT (after the exp chain) ----
            hT = [persist.tile([128, N], FP16, tag=f"hT{k}", name=f"hT{k}")
                  for k in range(KT)]
            for k in range(KT):
                for c in range(2):
                    nc.scalar.copy(
                        hT[k][:, c * 512:(c + 1) * 512],
                        hT32[k][:, c * 512:(c + 1) * 512],
                    )
            wk = []
            for k in range(KT):
                t = const.tile([128, FO], FP16, tag=f"W{k}", name=f"W{k}")
                nc.scalar.copy(t[:], wk32[k][:])
                wk.append(t)

            # ---- Wh_aug[jb][:, hh*65:+64] = (h @ W) block fp16, col 64 = 1
            # (after the exp/vz/bcast chain so its ACT copies don't delay
            # the exps; the PE matmuls trickle in behind the bcasts) ----
            wh_aug = [persist.tile([128, H * 65], FP16, tag=f"wha{j}",
                                   name=f"wha{j}")
                      for j in range(NB)]
            for jb in range(NB):
                ps = psS.tile([128, 512], F32, tag="ps")
                for k in range(KT):
                    nc.tensor.matmul(
                        ps[:], hT[k][:, jb * 128:(jb + 1) * 128], wk[k][:],
                        start=(k == 0), stop=(k == KT - 1),
                    )
                wv = wh_aug[jb][:].rearrange("p (h f) -> p h f", h=H)
                pv = ps[:].rearrange("p (h f) -> p h f", h=H)
                nc.scalar.copy(wv[:, :, 0:64], pv[:])
                nc.gpsimd.memset(wv[:, :, 64:65], 1.0)

            # ---- out staging: os_all[p, ib*512 + hh*64 + f] ----
            os_all = persist.tile([128, NB * FO], F32, tag="osall")

            def epi_a(hh, acc):
                # PSUM->SBUF (ACT, fp16, 1/16 scale keeps fp16 in range; the
                # final division acc/den is scale-invariant), transpose back
                # 4 blocks per PSUM tile.  Runs right after the head's
                # matmuls: the PE does the transposes while waiting for the
                # next head's x tiles (also keeps HAM warm).
                acc_sb = epi.tile([65, N], FP16, tag="accsb", name="acc_sb")
                for c in range(2):
                    nc.scalar.activation(
                        acc_sb[:, c * 512:(c + 1) * 512], acc[c][:],
                        AF.Copy, scale=1.0 / 16.0,
                    )
                tp4s = []
                for half in range(2):
                    tp4 = psS.tile([128, 264], FP16, tag="tp4", bufs=2,
                                   name="tp4")
                    for q in range(4):
                        ib = half * 4 + q
                        nc.tensor.transpose(
                            tp4[:, q * 66:q * 66 + 65],
                            acc_sb[:, ib * 128:(ib + 1) * 128],
                            identh[0:65, 0:65],
                        )
                    tp4s.append(tp4)
                return tp4s

            def epi_b(hh, tp4s):
                # strided reciprocal + scale-copies (+ output DMA halves).
                # Deferred one head so the DVE FIFO never stalls on it.
                last = hh == H - 1
                for half in range(2):
                    tp4 = tp4s[half]
                    t4v = tp4[:].rearrange("p (q f) -> p q f", f=66)
                    rec4 = epi.tile([128, 4], F32, tag="rec4", bufs=3,
                                    name="rec4")
                    r4v = rec4[:].rearrange("p (q o) -> p q o", o=1)
                    nc.vector.reciprocal(r4v[:], t4v[:, :, 64:65])
                    for q in range(4):
                        ib = half * 4 + q
                        dst = os_all[:, ib * FO + hh * FOH:
                                     ib * FO + (hh + 1) * FOH]
                        srcp = tp4[:, q * 66:q * 66 + 64]
                        if last and q % 2 == 1:
                            # spread the final head's scale-copies over DVE
                            # too, halving the serialized tail on ACT
                            nc.vector.tensor_scalar_mul(
                                dst, srcp, rec4[:, q:q + 1]
                            )
                        else:
                            nc.scalar.activation(
                                dst, srcp, AF.Copy, scale=rec4[:, q:q + 1],
                            )
                if hh == 3 or last:
                    lo = 0 if hh == 3 else 4 * FOH
                    hi = 4 * FOH if hh == 3 else FO
                    nc.sync.dma_start(
                        out_d[:, lo:hi].rearrange(
                            "(b p) f -> p b f", p=128),
                        os_all[:].rearrange(
                            "p (b f) -> p b f", b=NB)[:, :, lo:hi],
                    )

            # ---- main attention loop (epilogue tail deferred one head) ----
            prev = None
            for hh in range(H):
                acc = [psAcc.tile([65, 512], F32, tag=f"acc{c}",
                                  name=f"acc{c}")
                       for c in range(2)]
                for jb in range(NB):
                    x = xp.tile([128, N], FP16, tag="x")
                    nc.vector.tensor_scalar(
                        x[:], r_all[:, hh * N:(hh + 1) * N],
                        vz_sb[jb][:, 8 + hh:9 + hh],
                        vz_sb[jb][:, hh:hh + 1],
                        ALU.mult, ALU.max,
                    )
                    nc.vector.tensor_mul(x[:], x[:], adjB[jb][:])
                    for c in range(2):
                        nc.tensor.matmul(
                            acc[c][:],
                            wh_aug[jb][:, hh * 65:(hh + 1) * 65],
                            x[:, c * 512:(c + 1) * 512],
                            start=(jb == 0), stop=(jb == NB - 1),
                        )
                    if jb == 0 and hh + NPRE < H:
                        bcast(hh + NPRE)
                cur = (hh, epi_a(hh, acc))
                if prev is not None:
                    epi_b(*prev)
                prev = cur
            epi_b(*prev)

    if split:
        _split_sync_waits(nc)
    return nc


_NC_CACHE = None


def _get_nc():
    global _NC_CACHE
    if _NC_CACHE is None:
        _NC_CACHE = build_nc()
    return _NC_CACHE


def _prep_in_maps(h, adj, W, a):
    h = np.ascontiguousarray(h, dtype=np.float32)
    adj = np.ascontiguousarray(adj, dtype=np.int32)
    W = np.ascontiguousarray(W, dtype=np.float32)
    a = np.ascontiguousarray(a, dtype=np.float32)
    amat = np.zeros((FO, 3 * H), dtype=np.float32)
    for hh in range(H):
        amat[hh * FOH:(hh + 1) * FOH, hh] = a[hh, :FOH]
        amat[hh * FOH:(hh + 1) * FOH, H + hh] = a[hh, FOH:]
        amat[hh * FOH:(hh + 1) * FOH, 2 * H + hh] = ALPHA * a[hh, FOH:]
    wamat = (W @ amat).astype(np.float32)
    # e vectors on host: [bs, n, 3H] -> per core ES [H, n], ED2 [2H, n]
    ev = np.einsum("bnf,fe->ben", h, wamat).astype(np.float32)
    return [
        {
            "hT": np.ascontiguousarray(h[c].T),
            "adjT": np.ascontiguousarray(adj[c].T),
            "W": W,
            "ES": np.ascontiguousarray(ev[c, 0:H]),
            "ED2": np.ascontiguousarray(ev[c, H:3 * H]),
        }
        for c in range(N_CORES)
    ]


def run(h, adj, W, a, trace=False, **kw):
    nc = _get_nc()
    in_maps = _prep_in_maps(h, adj, W, a)
    res = run_bass_kernel_spmd(nc, in_maps, list(range(N_CORES)), trace=trace, **kw)
    out = np.stack([res.results[c]["out"] for c in range(N_CORES)], axis=0)
    return out.astype(np.float32), res


def kernel(h, adj, W, a):
    out, _ = run(h, adj, W, a)
    return out


# revision 27
# speedup vs baseline: 1.1703x; 1.1703x over previous
"""MultiHeadGAT layer on 8 trn2 NeuronCores, data-parallel over batch.

Per core (one batch element), exp(leaky_relu(e_src[i]+e_dst[j])) is
factored rank-1:  with u=exp(e_src), r=exp(-0.8 e_src), v=exp(e_dst),
z=exp(0.2 e_dst):

    exp(lrelu(s_ij)) = u_i * max(r_i z_j, v_j)

The row factor u_i cancels in the softmax, so the per-element work is

    S'[j,i] = adj[i,j] * max(r_i * z_j, v_j)

one fused DVE tensor_scalar (mult+max, fp16) + one DVE tensor_tensor
mask multiply (fp16 2x) per [128,1024] tile.  No full-size exp at all
(exp only on [8,1024] vectors; v and z share one exp via a host-side
WA extension).  The host passes h and adj pre-transposed, so no PE
transposes on the input side.  The AV matmul runs fp16 (1 cycle/row)
with a ones column appended to Wh so row 64 of the accumulator is the
softmax denominator.

r-broadcast tiles are produced by a transpose-mode matmul
(selector.T @ rv -> one fp16 PSUM bank) + one ACT copy, emitted two
heads ahead from inside the main loop so they slot into PE/ACT idle
gaps.  Epilogue is split: PSUM copies + transposes right after each
head's matmuls (fills PE gaps, keeps HAM warm); reciprocal + scaled
copies deferred one head so the DVE FIFO never stalls.  Output DMAs
go out in column halves after heads 3 and 7.
"""
import sys

sys.path.insert(0, "/opt/trn_rl_repo")

import numpy as np

import concourse.bass as bass
import concourse.mybir as mybir
import concourse.tile as tile
from concourse.bass_utils import run_bass_kernel_spmd
from concourse.masks import make_identity

F32 = mybir.dt.float32
FP16 = mybir.dt.float16
I32 = mybir.dt.int32
AF = mybir.ActivationFunctionType
ALU = mybir.AluOpType

N_CORES = 8
N = 1024
NB = 8          # row blocks of 128
FIN = 256
KT = 2          # FIN / 128
FO = 512        # heads * fo
H = 8
FOH = 64
ALPHA = 0.2
NPRE = 2        # heads whose r-broadcast is emitted before the main loop

_MAX_SYNC_WAITS = 1


def _split_sync_waits(nc, max_waits=_MAX_SYNC_WAITS):
    """This walrus build rejects instructions carrying more than one sync
    wait; hoist extras onto NOPs inserted just before, on the same engine."""
    uid = 0
    for f in nc.m.functions:
        for bb in f.blocks:
            out = []
            for inst in bb.instructions:
                si = getattr(inst, "sync_info", None)
                if si is not None and si.on_wait and len(si.on_wait) > max_waits:
                    waits = list(si.on_wait)
                    keep = waits[-max_waits:]
                    extra = waits[:-max_waits]
                    si.on_wait.clear()
                    si.on_wait.extend(keep)
                    while extra:
                        chunk, extra = extra[:max_waits], extra[max_waits:]
                        nop = mybir.InstNoOp(
                            name=f"waitsplit-{uid}",
                            engine=inst.engine,
                            sync_info=mybir.SyncInfo(
                                on_wait=list(chunk), on_update=[]
                            ),
                            bass_nofuse=True,
                        )
                        uid += 1
                        out.append(nop)
                out.append(inst)
            bb.instructions[:] = out


def build_nc(split=True):
    nc = bass.Bass()
    ht_d = nc.declare_dram_parameter("hT", [FIN, N], F32, isOutput=False)
    adjt_d = nc.declare_dram_parameter("adjT", [N, N], I32, isOutput=False)
    w_d = nc.declare_dram_parameter("W", [FIN, FO], F32, isOutput=False)
    es_d = nc.declare_dram_parameter("ES", [H, N], F32, isOutput=False)
    ed_d = nc.declare_dram_parameter("ED2", [2 * H, N], F32, isOutput=False)
    out_d = nc.declare_dram_parameter("out", [N, FO], F32, isOutput=True)

    with tile.TileContext(nc) as tc:
        with (
            tc.tile_pool(name="const", bufs=1) as const,
            tc.tile_pool(name="persist", bufs=1) as persist,
            tc.tile_pool(name="ld", bufs=4) as ld,
            tc.tile_pool(name="xp", bufs=8) as xp,
            tc.tile_pool(name="epi", bufs=2) as epi,
            tc.tile_pool(name="psS", bufs=2, space="PSUM") as psS,
            tc.tile_pool(name="psAcc", bufs=2, space="PSUM") as psAcc,
        ):
            ident = const.tile([128, 128], F32, tag="ident")
            make_identity(nc, ident[:])
            identh = const.tile([128, 128], FP16, tag="identh")
            nc.vector.tensor_copy(identh[:], ident[:])

            # ---- e vectors (host-computed h @ WA, tiny): load first ----
            e_src_t = const.tile([8, N], F32, tag="esT")
            ed2 = const.tile([16, N], F32, tag="ed2")
            nc.sync.dma_start(e_src_t[:], es_d[:, :])
            nc.sync.dma_start(ed2[:], ed_d[:, :])

            # ---- selector tiles for heads 0/1 before the adjB queue ----
            sel = []
            for hh in range(H):
                t = const.tile([8, 128], FP16, tag=f"sel{hh}", name=f"sel{hh}")
                sel.append(t)

            def mk_sel(hh):
                t = sel[hh]
                nc.gpsimd.memset(t[:], 0.0)
                nc.gpsimd.affine_select(
                    out=t[:], in_=t[:], pattern=[[0, 128]],
                    compare_op=ALU.not_equal, fill=1.0,
                    base=-hh, channel_multiplier=1,
                )

            for hh in range(NPRE):
                mk_sel(hh)

            # ---- adj.T (int32 DRAM) -> fp16 SBUF via SWDGE cast DMA ----
            adjB = [persist.tile([128, N], FP16, tag=f"adjB{j}",
                                 name=f"adjB{j}")
                    for j in range(NB)]
            for jb in range(NB):
                nc.gpsimd.dma_start(
                    adjB[jb][:], adjt_d[jb * 128:(jb + 1) * 128, :]
                )
            for hh in range(NPRE, H):
                mk_sel(hh)

            # ---- hT (fp32 DRAM, pre-transposed on host): sync load,
            # cast to fp16 on ACT (off the DVE FIFO) ----
            hT32 = [persist.tile([128, N], F32, tag=f"hT32_{k}",
                                 name=f"hT32_{k}")
                    for k in range(KT)]
            for k in range(KT):
                for c in range(2):
                    nc.sync.dma_start(
                        hT32[k][:, c * 512:(c + 1) * 512],
                        ht_d[k * 128:(k + 1) * 128, c * 512:(c + 1) * 512],
                    )
            wk32 = []
            for k in range(KT):
                t32 = ld.tile([128, FO], F32, tag="w32", name=f"w32_{k}")
                nc.sync.dma_start(t32[:], w_d[k * 128:(k + 1) * 128, :])
                wk32.append(t32)

            # ---- derived exp vectors ----
            # rv_t[hh, i] = exp(-0.8 * e_src[hh, i])        (fp16)
            rv_t = const.tile([8, N], FP16, tag="rvT")
            nc.scalar.activation(rv_t[:], e_src_t[:], AF.Exp, scale=-0.8)
            # vzexp rows 0:8 = v = exp(e_dst); rows 8:16 = z = exp(0.2 e_dst)
            vzexp = const.tile([16, N], FP16, tag="vzexp")
            nc.scalar.activation(vzexp[:], ed2[:], AF.Exp)

            # ---- vz_sb[jb][p, 0:8]=v_h(j), [p, 8:16]=z_h(j)  (f32) ----
            vz_sb = [persist.tile([128, 16], F32, tag=f"vz{j}", name=f"vz{j}")
                     for j in range(NB)]
            for jb in range(NB):
                tp = psS.tile([128, 512], FP16, tag="ps")
                nc.tensor.transpose(
                    tp[:, 0:16], vzexp[:, jb * 128:(jb + 1) * 128],
                    identh[0:16, 0:16],
                )
                nc.vector.tensor_copy(vz_sb[jb][:], tp[:, 0:16])

            # ---- r-broadcast: r_all[p, hh*N+i] = rv_t[hh, i].
            # One transpose-mode matmul (sel.T @ rv -> fp16 PSUM, one bank)
            # + one ACT copy per head.  Heads 0..NPRE-1 up front; head hh+2
            # emitted from inside head hh's loop body. ----
            r_all = persist.tile([128, H * N], FP16, tag="rall")

            def bcast(hh):
                for c in range(2):
                    rb = psS.tile([128, 512], F32, tag="ps", name="rb")
                    nc.tensor.matmul(
                        rb[:], sel[hh][:], rv_t[:, c * 512:(c + 1) * 512],
                        start=True, stop=True,
                    )
                    nc.scalar.copy(
                        r_all[:, hh * N + c * 512:hh * N + (c + 1) * 512],
                        rb[:],
                    )

            for hh in range(NPRE):
                bcast(hh)

            # ---- hT / W fp16 casts on ACT (after the exp chain) ----
            hT = [persist.tile([128, N], FP16, tag=f"hT{k}", name=f"hT{k}")
                  for k in range(KT)]
            for k in range(KT):
                for c in range(2):
                    nc.scalar.copy(
                        hT[k][:, c * 512:(c + 1) * 512],
                        hT32[k][:, c * 512:(c + 1) * 512],
                    )
            wk = []
            for k in range(KT):
                t = const.tile([128, FO], FP16, tag=f"W{k}", name=f"W{k}")
                nc.scalar.copy(t[:], wk32[k][:])
                wk.append(t)

            # ---- Wh_aug[jb][:, hh*65:+64] = (h @ W) block fp16, col 64 = 1
            # (after the exp/vz/bcast chain so its ACT copies don't delay
            # the exps; the PE matmuls trickle in behind the bcasts) ----
            wh_aug = [persist.tile([128, H * 65], FP16, tag=f"wha{j}",
                                   name=f"wha{j}")
                      for j in range(NB)]
            for jb in range(NB):
                ps = psS.tile([128, 512], F32, tag="ps")
                for k in range(KT):
                    nc.tensor.matmul(
                        ps[:], hT[k][:, jb * 128:(jb + 1) * 128], wk[k][:],
                        start=(k == 0), stop=(k == KT - 1),
                    )
                wv = wh_aug[jb][:].rearrange("p (h f) -> p h f", h=H)
                pv = ps[:].rearrange("p (h f) -> p h f", h=H)
                nc.scalar.copy(wv[:, :, 0:64], pv[:])
                nc.gpsimd.memset(wv[:, :, 64:65], 1.0)

            # ---- out staging: out_sb[ib][:, hh*64+f] ----
            out_sb = [persist.tile([128, FO], F32, tag=f"os{i}", name=f"os{i}")
                      for i in range(NB)]

            def epi_a(hh, acc):
                # PSUM->SBUF (ACT, fp16, 1/16 scale keeps fp16 in range; the
                # final division acc/den is scale-invariant), transpose back
                # 4 blocks per PSUM tile.  Runs right after the head's
                # matmuls: the PE does the transposes while waiting for the
                # next head's x tiles (also keeps HAM warm).
                acc_sb = epi.tile([65, N], FP16, tag="accsb", name="acc_sb")
                for c in range(2):
                    nc.scalar.activation(
                        acc_sb[:, c * 512:(c + 1) * 512], acc[c][:],
                        AF.Copy, scale=1.0 / 16.0,
                    )
                tp4s = []
                for half in range(2):
                    tp4 = psS.tile([128, 264], FP16, tag="tp4", bufs=2,
                                   name="tp4")
                    for q in range(4):
                        ib = half * 4 + q
                        nc.tensor.transpose(
                            tp4[:, q * 66:q * 66 + 65],
                            acc_sb[:, ib * 128:(ib + 1) * 128],
                            identh[0:65, 0:65],
                        )
                    tp4s.append(tp4)
                return tp4s

            def epi_b(hh, tp4s):
                # strided reciprocal + scale-copies (+ output DMA halves).
                # Deferred one head so the DVE FIFO never stalls on it.
                last = hh == H - 1
                for half in range(2):
                    tp4 = tp4s[half]
                    t4v = tp4[:].rearrange("p (q f) -> p q f", f=66)
                    rec4 = epi.tile([128, 4], F32, tag="rec4", bufs=3,
                                    name="rec4")
                    r4v = rec4[:].rearrange("p (q o) -> p q o", o=1)
                    nc.vector.reciprocal(r4v[:], t4v[:, :, 64:65])
                    for q in range(4):
                        ib = half * 4 + q
                        dst = out_sb[ib][:, hh * FOH:(hh + 1) * FOH]
                        srcp = tp4[:, q * 66:q * 66 + 64]
                        if last and q % 2 == 1:
                            # spread the final head's scale-copies over DVE
                            # too, halving the serialized tail on ACT
                            nc.vector.tensor_scalar_mul(
                                dst, srcp, rec4[:, q:q + 1]
                            )
                        else:
                            nc.scalar.activation(
                                dst, srcp, AF.Copy, scale=rec4[:, q:q + 1],
                            )
                        if hh == 3:
                            nc.sync.dma_start(
                                out_d[ib * 128:(ib + 1) * 128, 0:4 * FOH],
                                out_sb[ib][:, 0:4 * FOH],
                            )
                        elif last:
                            nc.sync.dma_start(
                                out_d[ib * 128:(ib + 1) * 128, 4 * FOH:FO],
                                out_sb[ib][:, 4 * FOH:FO],
                            )

            # ---- main attention loop (epilogue tail deferred one head) ----
            prev = None
            for hh in range(H):
                acc = [psAcc.tile([65, 512], F32, tag=f"acc{c}",
                                  name=f"acc{c}")
                       for c in range(2)]
                for jb in range(NB):
                    x = xp.tile([128, N], FP16, tag="x")
                    nc.vector.tensor_scalar(
                        x[:], r_all[:, hh * N:(hh + 1) * N],
                        vz_sb[jb][:, 8 + hh:9 + hh],
                        vz_sb[jb][:, hh:hh + 1],
                        ALU.mult, ALU.max,
                    )
                    nc.vector.tensor_mul(x[:], x[:], adjB[jb][:])
                    for c in range(2):
                        nc.tensor.matmul(
                            acc[c][:],
                            wh_aug[jb][:, hh * 65:(hh + 1) * 65],
                            x[:, c * 512:(c + 1) * 512],
                            start=(jb == 0), stop=(jb == NB - 1),
                        )
                    if jb == 0 and hh + NPRE < H:
                        bcast(hh + NPRE)
                cur = (hh, epi_a(hh, acc))
                if prev is not None:
                    epi_b(*prev)
                prev = cur
            epi_b(*prev)

    if split:
        _split_sync_waits(nc)
    return nc


_NC_CACHE = None


def _get_nc():
    global _NC_CACHE
    if _NC_CACHE is None:
        _NC_CACHE = build_nc()
    return _NC_CACHE


def _prep_in_maps(h, adj, W, a):
    h = np.ascontiguousarray(h, dtype=np.float32)
    adj = np.ascontiguousarray(adj, dtype=np.int32)
    W = np.ascontiguousarray(W, dtype=np.float32)
    a = np.ascontiguousarray(a, dtype=np.float32)
    amat = np.zeros((FO, 3 * H), dtype=np.float32)
    for hh in range(H):
        amat[hh * FOH:(hh + 1) * FOH, hh] = a[hh, :FOH]
        amat[hh * FOH:(hh + 1) * FOH, H + hh] = a[hh, FOH:]
        amat[hh * FOH:(hh + 1) * FOH, 2 * H + hh] = ALPHA * a[hh, FOH:]
    wamat = (W @ amat).astype(np.float32)
    # e vectors on host: [bs, n, 3H] -> per core ES [H, n], ED2 [2H, n]
    ev = np.einsum("bnf,fe->ben", h, wamat).astype(np.float32)
    return [
        {
            "hT": np.ascontiguousarray(h[c].T),
            "adjT": np.ascontiguousarray(adj[c].T),
            "W": W,
            "ES": np.ascontiguousarray(ev[c, 0:H]),
            "ED2": np.ascontiguousarray(ev[c, H:3 * H]),
        }
        for c in range(N_CORES)
    ]


def run(h, adj, W, a, trace=False, **kw):
    nc = _get_nc()
    in_maps = _prep_in_maps(h, adj, W, a)
    res = run_bass_kernel_spmd(nc, in_maps, list(range(N_CORES)), trace=trace, **kw)
    out = np.stack([res.results[c]["out"] for c in range(N_CORES)], axis=0)
    return out.astype(np.float32), res


def kernel(h, adj, W, a):
    out, _ = run(h, adj, W, a)
    return out


# revision 29
# speedup vs baseline: 1.2604x; 1.0771x over previous
"""MultiHeadGAT layer on 8 trn2 NeuronCores, data-parallel over batch.

Per core (one batch element), exp(leaky_relu(e_src[i]+e_dst[j])) is
factored rank-1:  with u=exp(e_src), r=exp(-0.8 e_src), v=exp(e_dst),
z=exp(0.2 e_dst):

    exp(lrelu(s_ij)) = u_i * max(r_i z_j, v_j)

The row factor u_i cancels in the softmax, so the per-element work is

    S'[j,i] = adj[i,j] * max(r_i * z_j, v_j)

one fused DVE tensor_scalar (mult+max, fp16) + one DVE tensor_tensor
mask multiply (fp16 2x) per [128,1024] tile.  No full-size exp at all
(exp only on [8,1024] vectors; v and z share one exp via a host-side
WA extension).  The host passes h and adj pre-transposed, so no PE
transposes on the input side.  The AV matmul runs fp16 (1 cycle/row)
with a ones column appended to Wh so row 64 of the accumulator is the
softmax denominator.

r-broadcast tiles are produced by a transpose-mode matmul
(selector.T @ rv -> one fp16 PSUM bank) + one ACT copy, emitted two
heads ahead from inside the main loop so they slot into PE/ACT idle
gaps.  Epilogue is split: PSUM copies + transposes right after each
head's matmuls (fills PE gaps, keeps HAM warm); reciprocal + scaled
copies deferred one head so the DVE FIFO never stalls.  Output DMAs
go out in column halves after heads 3 and 7.
"""
import sys

sys.path.insert(0, "/opt/trn_rl_repo")

import numpy as np

import concourse.bass as bass
import concourse.mybir as mybir
import concourse.tile as tile
from concourse.bass_utils import run_bass_kernel_spmd
from concourse.masks import make_identity

F32 = mybir.dt.float32
FP16 = mybir.dt.float16
I32 = mybir.dt.int32
AF = mybir.ActivationFunctionType
ALU = mybir.AluOpType

N_CORES = 8
N = 1024
NB = 8          # row blocks of 128
FIN = 256
KT = 2          # FIN / 128
FO = 512        # heads * fo
H = 8
FOH = 64
ALPHA = 0.2
NPRE = 2        # heads whose r-broadcast is emitted before the main loop

_MAX_SYNC_WAITS = 1


def _split_sync_waits(nc, max_waits=_MAX_SYNC_WAITS):
    """This walrus build rejects instructions carrying more than one sync
    wait; hoist extras onto NOPs inserted just before, on the same engine."""
    uid = 0
    for f in nc.m.functions:
        for bb in f.blocks:
            out = []
            for inst in bb.instructions:
                si = getattr(inst, "sync_info", None)
                if si is not None and si.on_wait and len(si.on_wait) > max_waits:
                    waits = list(si.on_wait)
                    keep = waits[-max_waits:]
                    extra = waits[:-max_waits]
                    si.on_wait.clear()
                    si.on_wait.extend(keep)
                    while extra:
                        chunk, extra = extra[:max_waits], extra[max_waits:]
                        nop = mybir.InstNoOp(
                            name=f"waitsplit-{uid}",
                            engine=inst.engine,
                            sync_info=mybir.SyncInfo(
                                on_wait=list(chunk), on_update=[]
                            ),
                            bass_nofuse=True,
                        )
                        uid += 1
                        out.append(nop)
                out.append(inst)
            bb.instructions[:] = out


def build_nc(split=True):
    nc = bass.Bass()
    ht_d = nc.declare_dram_parameter("hT", [FIN, N], FP16, isOutput=False)
    adjt_d = nc.declare_dram_parameter("adjT", [N, N], mybir.dt.uint8,
                                   isOutput=False)
    w_d = nc.declare_dram_parameter("W", [FIN, FO], FP16, isOutput=False)
    es_d = nc.declare_dram_parameter("ES", [H, N], F32, isOutput=False)
    ed_d = nc.declare_dram_parameter("ED2", [2 * H, N], F32, isOutput=False)
    out_d = nc.declare_dram_parameter("out", [N, FO], F32, isOutput=True)

    with tile.TileContext(nc) as tc:
        with (
            tc.tile_pool(name="const", bufs=1) as const,
            tc.tile_pool(name="persist", bufs=1) as persist,
            tc.tile_pool(name="ld", bufs=4) as ld,
            tc.tile_pool(name="xp", bufs=8) as xp,
            tc.tile_pool(name="epi", bufs=2) as epi,
            tc.tile_pool(name="psS", bufs=2, space="PSUM") as psS,
            tc.tile_pool(name="psAcc", bufs=2, space="PSUM") as psAcc,
        ):
            ident = const.tile([128, 128], F32, tag="ident")
            make_identity(nc, ident[:])
            identh = const.tile([128, 128], FP16, tag="identh")
            nc.vector.tensor_copy(identh[:], ident[:])

            # ---- e vectors (host-computed h @ WA, tiny): load first ----
            e_src_t = const.tile([8, N], F32, tag="esT")
            ed2 = const.tile([16, N], F32, tag="ed2")
            nc.sync.dma_start(e_src_t[:], es_d[:, :])
            nc.sync.dma_start(ed2[:], ed_d[:, :])

            # ---- selector tiles for heads 0/1 before the adjB queue ----
            sel = []
            for hh in range(H):
                t = const.tile([8, 128], FP16, tag=f"sel{hh}", name=f"sel{hh}")
                sel.append(t)

            def mk_sel(hh):
                t = sel[hh]
                nc.gpsimd.memset(t[:], 0.0)
                nc.gpsimd.affine_select(
                    out=t[:], in_=t[:], pattern=[[0, 128]],
                    compare_op=ALU.not_equal, fill=1.0,
                    base=-hh, channel_multiplier=1,
                )

            for hh in range(NPRE):
                mk_sel(hh)

            # ---- adj.T (int32 DRAM) -> fp16 SBUF via SWDGE cast DMA ----
            adjB = [persist.tile([128, N], FP16, tag=f"adjB{j}",
                                 name=f"adjB{j}")
                    for j in range(NB)]
            for jb in range(NB):
                nc.gpsimd.dma_start(
                    adjB[jb][:], adjt_d[jb * 128:(jb + 1) * 128, :]
                )
            for hh in range(NPRE, H):
                mk_sel(hh)

            # ---- hT / W (fp16 DRAM, host-prepared): direct loads ----
            hT = [persist.tile([128, N], FP16, tag=f"hT{k}", name=f"hT{k}")
                  for k in range(KT)]
            for k in range(KT):
                nc.sync.dma_start(hT[k][:], ht_d[k * 128:(k + 1) * 128, :])
            wk = []
            for k in range(KT):
                t = const.tile([128, FO], FP16, tag=f"W{k}", name=f"W{k}")
                nc.sync.dma_start(t[:], w_d[k * 128:(k + 1) * 128, :])
                wk.append(t)

            # ---- derived exp vectors ----
            # rv_t[hh, i] = exp(-0.8 * e_src[hh, i])        (fp16)
            rv_t = const.tile([8, N], FP16, tag="rvT")
            nc.scalar.activation(rv_t[:], e_src_t[:], AF.Exp, scale=-0.8)
            # vzexp rows 0:8 = v = exp(e_dst); rows 8:16 = z = exp(0.2 e_dst)
            vzexp = const.tile([16, N], FP16, tag="vzexp")
            nc.scalar.activation(vzexp[:], ed2[:], AF.Exp)

            # ---- vz_sb[jb][p, 0:8]=v_h(j), [p, 8:16]=z_h(j)  (f32) ----
            vz_sb = [persist.tile([128, 16], F32, tag=f"vz{j}", name=f"vz{j}")
                     for j in range(NB)]
            for jb in range(NB):
                tp = psS.tile([128, 512], FP16, tag="ps")
                nc.tensor.transpose(
                    tp[:, 0:16], vzexp[:, jb * 128:(jb + 1) * 128],
                    identh[0:16, 0:16],
                )
                nc.vector.tensor_copy(vz_sb[jb][:], tp[:, 0:16])

            # ---- r-broadcast: r_all[p, hh*N+i] = rv_t[hh, i].
            # One transpose-mode matmul (sel.T @ rv -> fp16 PSUM, one bank)
            # + one ACT copy per head.  Heads 0..NPRE-1 up front; head hh+2
            # emitted from inside head hh's loop body. ----
            r_b = [persist.tile([128, N], FP16, tag=f"rb{hh}",
                                name=f"rb{hh}")
                   for hh in range(H)]

            def bcast(hh):
                for c in range(2):
                    rb = psS.tile([128, 512], F32, tag="ps", name="rb")
                    nc.tensor.matmul(
                        rb[:], sel[hh][:], rv_t[:, c * 512:(c + 1) * 512],
                        start=True, stop=True,
                    )
                    nc.scalar.copy(
                        r_b[hh][:, c * 512:(c + 1) * 512], rb[:],
                    )

            for hh in range(NPRE):
                bcast(hh)

            # ---- Wh_aug[jb][:, hh*65:+64] = (h @ W) block fp16, col 64 = 1
            # (after the exp/vz/bcast chain so its ACT copies don't delay
            # the exps; the PE matmuls trickle in behind the bcasts) ----
            wh_aug = [persist.tile([128, H * 65], FP16, tag=f"wha{j}",
                                   name=f"wha{j}")
                      for j in range(NB)]
            for jb in range(NB):
                ps = psS.tile([128, 512], F32, tag="ps")
                for k in range(KT):
                    nc.tensor.matmul(
                        ps[:], hT[k][:, jb * 128:(jb + 1) * 128], wk[k][:],
                        start=(k == 0), stop=(k == KT - 1),
                    )
                wv = wh_aug[jb][:].rearrange("p (h f) -> p h f", h=H)
                pv = ps[:].rearrange("p (h f) -> p h f", h=H)
                nc.scalar.copy(wv[:, :, 0:64], pv[:])
                nc.gpsimd.memset(wv[:, :, 64:65], 1.0)

            # ---- out staging: out_sb[ib][:, hh*64+f] ----
            out_sb = [persist.tile([128, FO], F32, tag=f"os{i}", name=f"os{i}")
                      for i in range(NB)]

            def epi_a(hh, acc):
                # PSUM->SBUF (ACT, fp16, 1/16 scale keeps fp16 in range; the
                # final division acc/den is scale-invariant), transpose back
                # 4 blocks per PSUM tile.  Runs right after the head's
                # matmuls: the PE does the transposes while waiting for the
                # next head's x tiles (also keeps HAM warm).
                acc_sb = epi.tile([65, N], FP16, tag="accsb", name="acc_sb")
                for c in range(2):
                    nc.scalar.activation(
                        acc_sb[:, c * 512:(c + 1) * 512], acc[c][:],
                        AF.Copy, scale=1.0 / 16.0,
                    )
                tp4s = []
                for half in range(2):
                    tp4 = psS.tile([128, 264], FP16, tag="tp4", bufs=2,
                                   name="tp4")
                    for q in range(4):
                        ib = half * 4 + q
                        nc.tensor.transpose(
                            tp4[:, q * 66:q * 66 + 65],
                            acc_sb[:, ib * 128:(ib + 1) * 128],
                            identh[0:65, 0:65],
                        )
                    tp4s.append(tp4)
                return tp4s

            def epi_b(hh, tp4s):
                # strided reciprocal + scale-copies (+ output DMA halves).
                # Deferred one head so the DVE FIFO never stalls on it.
                last = hh == H - 1
                for half in range(2):
                    tp4 = tp4s[half]
                    t4v = tp4[:].rearrange("p (q f) -> p q f", f=66)
                    rec4 = epi.tile([128, 4], F32, tag="rec4", bufs=3,
                                    name="rec4")
                    r4v = rec4[:].rearrange("p (q o) -> p q o", o=1)
                    nc.vector.reciprocal(r4v[:], t4v[:, :, 64:65])
                    for q in range(4):
                        ib = half * 4 + q
                        dst = out_sb[ib][:, hh * FOH:(hh + 1) * FOH]
                        srcp = tp4[:, q * 66:q * 66 + 64]
                        if last:
                            # final head: DVE (idle at the tail) does the
                            # scale-copies, ACT is the slower path here
                            nc.vector.tensor_scalar_mul(
                                dst, srcp, rec4[:, q:q + 1]
                            )
                        else:
                            nc.scalar.activation(
                                dst, srcp, AF.Copy, scale=rec4[:, q:q + 1],
                            )
                        if hh == 3:
                            nc.sync.dma_start(
                                out_d[ib * 128:(ib + 1) * 128, 0:4 * FOH],
                                out_sb[ib][:, 0:4 * FOH],
                            )
                        elif last:
                            eng = nc.sync if ib % 2 == 0 else nc.gpsimd
                            eng.dma_start(
                                out_d[ib * 128:(ib + 1) * 128, 4 * FOH:FO],
                                out_sb[ib][:, 4 * FOH:FO],
                            )

            # ---- main attention loop (epilogue tail deferred one head) ----
            prev = None
            for hh in range(H):
                acc = [psAcc.tile([65, 512], F32, tag=f"acc{c}",
                                  name=f"acc{c}")
                       for c in range(2)]
                for jb in range(NB):
                    x = xp.tile([128, N], FP16, tag="x")
                    nc.vector.tensor_scalar(
                        x[:], r_b[hh][:],
                        vz_sb[jb][:, 8 + hh:9 + hh],
                        vz_sb[jb][:, hh:hh + 1],
                        ALU.mult, ALU.max,
                    )
                    nc.vector.tensor_mul(x[:], x[:], adjB[jb][:])
                    for c in range(2):
                        nc.tensor.matmul(
                            acc[c][:],
                            wh_aug[jb][:, hh * 65:(hh + 1) * 65],
                            x[:, c * 512:(c + 1) * 512],
                            start=(jb == 0), stop=(jb == NB - 1),
                        )
                    if jb == 0 and hh + NPRE < H:
                        bcast(hh + NPRE)
                cur = (hh, epi_a(hh, acc))
                if prev is not None:
                    epi_b(*prev)
                prev = cur
            epi_b(*prev)

    if split:
        _split_sync_waits(nc)
    return nc


_NC_CACHE = None


def _get_nc():
    global _NC_CACHE
    if _NC_CACHE is None:
        _NC_CACHE = build_nc()
    return _NC_CACHE


def _prep_in_maps(h, adj, W, a):
    h = np.ascontiguousarray(h, dtype=np.float32)
    adj = np.ascontiguousarray(adj, dtype=np.int32)
    W = np.ascontiguousarray(W, dtype=np.float32)
    a = np.ascontiguousarray(a, dtype=np.float32)
    amat = np.zeros((FO, 3 * H), dtype=np.float32)
    for hh in range(H):
        amat[hh * FOH:(hh + 1) * FOH, hh] = a[hh, :FOH]
        amat[hh * FOH:(hh + 1) * FOH, H + hh] = a[hh, FOH:]
        amat[hh * FOH:(hh + 1) * FOH, 2 * H + hh] = ALPHA * a[hh, FOH:]
    wamat = (W @ amat).astype(np.float32)
    # e vectors on host: [bs, n, 3H] -> per core ES [H, n], ED2 [2H, n]
    ev = np.einsum("bnf,fe->ben", h, wamat).astype(np.float32)
    return [
        {
            "hT": np.ascontiguousarray(h[c].T.astype(np.float16)),
            "adjT": np.ascontiguousarray(adj[c].T.astype(np.uint8)),
            "W": np.ascontiguousarray(W.astype(np.float16)),
            "ES": np.ascontiguousarray(ev[c, 0:H]),
            "ED2": np.ascontiguousarray(ev[c, H:3 * H]),
        }
        for c in range(N_CORES)
    ]


def run(h, adj, W, a, trace=False, **kw):
    nc = _get_nc()
    in_maps = _prep_in_maps(h, adj, W, a)
    res = run_bass_kernel_spmd(nc, in_maps, list(range(N_CORES)), trace=trace, **kw)
    out = np.stack([res.results[c]["out"] for c in range(N_CORES)], axis=0)
    return out.astype(np.float32), res


def kernel(h, adj, W, a):
    out, _ = run(h, adj, W, a)
    return out
